# revision 34
# baseline (speedup 1.0000x reference)
"""Trainium2 Bass kernel for nn_BasisJastrow.

Math (per batch element b):
    J_b = (1/P) * sum_{i<j} chi_j^T C chi_i ,   P = N(N-1)/2, C = coeff.reshape(Nb, Nb)

Device decomposition (per core; data-parallel over the batch axis).
The 48 per-core batches are split into two halves on disjoint SBUF partition
ranges (A = batches 0..23 on partitions 0:64, B = batches 24..47 on 64:128).
All matmul tensors are bf16 (PSUM accumulation stays f32); tolerance is 2e-2.

  layout  Xl[n + 64*half, (b,u)]   n=64 on partitions, 24 batches * 32 per half
  phase 1 S  = Lt.T @ Xl           exclusive prefix sums over particles (PE,
                                   concurrent 64x64 quadrants per half,
                                   4 x 192-column sub-chunks)
  phase 2 Q_p = Xp.T @ Sp          2-batch cross-Gram [64,64] per half; diag
                                   32x32 blocks are G_b, off-diag is garbage
  phase 3 r_p[q] = sum_f Q_p[q,f] * CD2[q,f]  DVE multiplies each 3-pair
                                   bank by the mask (PSUM -> SBUF bf16) and
                                   reduces bank pairs with tensor_reduce
  phase 4 J = id4.T @ R            partition-block reduction of r (PE)

Engine roles: Sync = ring A DMA (half A + const pack) + output DMA.
Scalar = ring B DMA + the four S casts (a dummy activation preloads the
1.3us ACT_TABLE during the DMA window).  GpSimd = Lt/id4 generation.
DVE = mask mults + reductions + J copy.  PE = warmup + phases 1/2/4.

DMA cost is per dst-partition line (~10ns/line), so each ring loads its
64-partition half in ONE transfer; column chunking would double the time.

Raw Bass (explicit engine blocks + semaphores): the walrus build in this
container rejects any instruction carrying more than one sync wait, which
rules out Tile's generated sem placement; raw Bass emits waits standalone.
"""

import sys

for _p in ("/opt/trn_rl_repo",):
    if _p not in sys.path:
        sys.path.insert(0, _p)

import numpy as np

import concourse.bass as bass
from concourse import mybir
from concourse.bass_utils import run_bass_kernel_spmd

B, N, Nb = 384, 64, 32
NCORES = 8
BS = B // NCORES            # 48 batches per core
HB = BS // 2                # 24 batches per half
NP = HB // 2                # 12 concurrent gram pairs
NPAIR = N * (N - 1) // 2    # 2016
F32 = mybir.dt.float32

MM_DTYPE = "bf16"  # "f32" | "bf16"
USE_BF16 = MM_DTYPE == "bf16"
MM_DT = mybir.dt.bfloat16 if USE_BF16 else mybir.dt.float32

FREE = HB * Nb              # 768 free columns per half
NSUB = 6                    # phase-1 / cast sub-chunks (128 cols each)
SUB = FREE // NSUB
NBANK = 4                   # gram psum banks
PPB = 3                     # pairs per bank
N_WARM = 6                  # gapless PE warmup matmuls during the DMA window
WARM_COLS = 192

CP_COLS = 192               # const pack = CD2 tiled 3x (one bank's pairs)


def build_nc() -> bass.Bass:
    nc = bass.Bass()

    x_d = nc.dram_tensor("x", [128, FREE], MM_DT, kind="ExternalInput")
    cp_d = nc.dram_tensor("cp", [128, CP_COLS], F32, kind="ExternalInput")
    j_d = nc.dram_tensor("j", [4, NP], F32, kind="ExternalOutput")

    from contextlib import ExitStack

    with ExitStack() as ctx:
        ctx.enter_context(
            nc.allow_low_precision("bf16 pair sums stay within the 2e-2 gate")
        )
        x_sb = ctx.enter_context(nc.sbuf_tensor("x_sb", [128, FREE], MM_DT))
        s_sb = ctx.enter_context(nc.sbuf_tensor("s_sb", [128, FREE], MM_DT))
        cp_sb = ctx.enter_context(nc.sbuf_tensor("cp_sb", [128, CP_COLS], F32))
        w_sb = ctx.enter_context(
            nc.sbuf_tensor("w_sb", [128, 4 + WARM_COLS], MM_DT)
        )
        lt_sb = ctx.enter_context(nc.sbuf_tensor("lt_sb", [128, N], MM_DT))
        id4_sb = ctx.enter_context(nc.sbuf_tensor("id4_sb", [128, 4], MM_DT))
        dum_sb = ctx.enter_context(nc.sbuf_tensor("dum_sb", [128, 1], MM_DT))
        e_sb = ctx.enter_context(
            nc.sbuf_tensor("e_sb", [128, NBANK, PPB * 64], MM_DT)
        )
        r_sb = ctx.enter_context(nc.sbuf_tensor("r_sb", [128, NP], MM_DT))
        j_sb = ctx.enter_context(nc.sbuf_tensor("j_sb", [4, NP], F32))
        s_ps = [
            ctx.enter_context(
                nc.psum_tensor(f"s_ps{h}", [128, FREE // 2], F32)
            )
            for h in range(2)
        ]
        QW = PPB * 64
        q_ps = [
            ctx.enter_context(
                nc.psum_tensor(
                    f"q_ps{k}", [128, QW + (16 if k == 0 else 0)], F32
                )
            )
            for k in range(NBANK)
        ]

        def sps(cs):
            h, off = cs.start // (FREE // 2), cs.start % (FREE // 2)
            return s_ps[h][:, off : off + (cs.stop - cs.start)]
        jw = None  # set below once q_ps exists
        dma_a = ctx.enter_context(nc.semaphore("dma_a"))
        dma_b = ctx.enter_context(nc.semaphore("dma_b"))
        dma_c = ctx.enter_context(nc.semaphore("dma_c"))
        dma_o = ctx.enter_context(nc.semaphore("dma_o"))
        pe = ctx.enter_context(nc.semaphore("pe"))
        pw = ctx.enter_context(nc.semaphore("pw"))
        dve = ctx.enter_context(nc.semaphore("dve"))
        sc = ctx.enter_context(nc.semaphore("sc"))
        gp_w = ctx.enter_context(nc.semaphore("gp_w"))
        block = ctx.enter_context(nc.Block())
        jw = q_ps[0][0:4, QW : QW + NP]

        lt = lt_sb[:]
        cd2r = cp_sb[:, 0:CP_COLS].rearrange("p (r f) -> p r f", r=PPB)

        # pe ledger: phase1 sub-chunks -> 1..NSUB; pair p done -> NSUB+1+p;
        # phase4 halves -> NSUB+NP+1, +2
        PAIR_DONE = lambda p: NSUB + 1 + p
        PH4_DONE = NSUB + NP + 1
        # dve ledger: m0,m1 -> 1,2; r01 -> 3; m2,m3 -> 4,5; r23 -> 6;
        # j halves -> 7,8
        JCOPY_DONE = 7
        # sc ledger: 1..NSUB = S casts

        @block.sync
        def _(sync):
            # ring A: half A (partitions 0:64) in one transfer, then consts
            sync.dma_start(out=x_sb[0:64, :], in_=x_d[0:64, :]).then_inc(
                dma_a, 16
            )
            sync.dma_start(out=cp_sb[:], in_=cp_d[:]).then_inc(dma_c, 16)
            # J leaves in two halves; the first DMA trigger (~0.9us) runs
            # while the DVE still works on banks 2/3
            for g in range(2):
                sync.wait_ge(sc, NSUB + 1 + g)
                cols = slice(2 * PPB * g, 2 * PPB * (g + 1))
                sync.dma_start(out=j_d[:, cols], in_=j_sb[:, cols]).then_inc(
                    dma_o, 16
                )

        @block.scalar
        def _(scalar):
            # ring B: half B (partitions 64:128) in one transfer
            scalar.dma_start(out=x_sb[64:128, :], in_=x_d[64:128, :]).then_inc(
                dma_b, 16
            )
            # dummy activation: hides the one-time ACT_TABLE_LOAD (~1.5us)
            # inside the DMA window so the S casts below start immediately
            scalar.wait_ge(gp_w, 1)
            scalar.copy(dum_sb[:], w_sb[:, 0:1])
            # S casts (PSUM f32 -> SBUF bf16), one per phase-1 sub-chunk
            for c in range(NSUB):
                cs = slice(c * SUB, (c + 1) * SUB)
                scalar.wait_ge(pe, c + 1)
                scalar.copy(s_sb[:, cs], sps(cs)).then_inc(sc, 1)
            # J copies (PSUM -> SBUF) per half, handed to Sync's out DMA
            for g in range(2):
                cols = slice(2 * PPB * g, 2 * PPB * (g + 1))
                scalar.wait_ge(pe, PH4_DONE + g)
                scalar.copy(j_sb[:, cols], jw[:, cols]).then_inc(sc, 1)

        @block.gpsimd
        def _(gpsimd):
            gpsimd.memset(w_sb[:], 1.0).then_inc(gp_w, 1)
            gpsimd.wait_ge(gp_w, 1)
            for h in range(2):
                hs = slice(h * 64, (h + 1) * 64)
                gpsimd.affine_select(
                    out=lt_sb[hs, :],
                    in_=w_sb[hs, 0:N],
                    pattern=[[1, N]],
                    compare_op=mybir.AluOpType.is_gt,
                    fill=0.0,
                    base=0,
                    channel_multiplier=-1,
                ).then_inc(gp_w, 1)
            # id4 for phase 4: col k = ones on partitions 32k:32k+32.
            # Disjoint 32-partition memsets (race checker + GpSimd window)
            for k in range(4):
                for blk in range(4):
                    inst = gpsimd.memset(
                        id4_sb[32 * blk : 32 * (blk + 1), k : k + 1],
                        1.0 if blk == k else 0.0,
                    )
            inst.then_inc(gp_w, 1)

        @block.tensor
        def _(tensor):
            # gapless PE warmups (disjoint s_ps scratch regions, so no
            # chain sems) keep the PE p-state high through the DMA window
            tensor.wait_ge(gp_w, 1)
            for w in range(N_WARM):
                if w < 4:
                    tgt = s_ps[w // 2][
                        0:4, (w % 2) * WARM_COLS : (w % 2 + 1) * WARM_COLS
                    ]
                else:
                    tgt = q_ps[w - 4][0:4, 0:WARM_COLS]
                inst = tensor.matmul(
                    tgt,
                    w_sb[0:64, 0:4],
                    w_sb[0:64, 4 : 4 + WARM_COLS],
                    start=True,
                    stop=True,
                )
            inst.then_inc(pw, 1)
            tensor.wait_ge(gp_w, 3)
            # order phase 1's s_ps writes after the warm scratch writes
            tensor.wait_ge(pw, 1)
            # phase 1: exclusive prefix sums, concurrent halves
            for c in range(NSUB):
                cs = slice(c * SUB, (c + 1) * SUB)
                if c == 0:
                    tensor.wait_ge(dma_a, 16)
                sp = sps(cs)
                tensor.matmul(
                    sp[0:64, :],
                    lt[0:64, :],
                    x_sb[0:64, cs],
                    start=True,
                    stop=True,
                    tile_position=(0, 0),
                )
                if c == 0:
                    tensor.wait_ge(dma_b, 16)
                tensor.matmul(
                    sp[64:128, :],
                    lt[64:128, :],
                    x_sb[64:128, cs],
                    start=True,
                    stop=True,
                    tile_position=(64, 64),
                ).then_inc(pe, 1)
            # phase 2: 2-batch cross-Grams, concurrent halves
            prev_need = 0
            for p in range(NP):
                ps_ = slice(p * 64, (p + 1) * 64)
                need = ((p + 1) * 64 - 1) // SUB + 1
                if need > prev_need:
                    tensor.wait_ge(sc, need)
                    prev_need = need
                q = q_ps[p // PPB][:, (p % PPB) * 64 : (p % PPB + 1) * 64]
                tensor.matmul(
                    q[0:64, :],
                    x_sb[0:64, ps_],
                    s_sb[0:64, ps_],
                    start=True,
                    stop=True,
                    tile_position=(0, 0),
                )
                tensor.matmul(
                    q[64:128, :],
                    x_sb[64:128, ps_],
                    s_sb[64:128, ps_],
                    start=True,
                    stop=True,
                    tile_position=(64, 64),
                ).then_inc(pe, 1)
            # phase 4: partition-block reduction of r columns, split in
            # two so the first half overlaps the rest of the DVE chain
            tensor.wait_ge(gp_w, 4)
            for g in range(2):
                cols = slice(2 * PPB * g, 2 * PPB * (g + 1))
                tensor.wait_ge(dve, 3 * (g + 1))
                tensor.matmul(
                    jw[:, cols],
                    id4_sb[:],
                    r_sb[:, cols],
                    start=True,
                    stop=True,
                ).then_inc(pe, 1)

        @block.vector
        def _(vector):
            # phase 3 mask-mults: e = Q * CD2 per 3-pair bank (PSUM->SBUF),
            # with a fused 2-bank reduce after each pair of mults
            vector.wait_ge(dma_c, 16)
            for g in range(2):
                for k in (2 * g, 2 * g + 1):
                    vector.wait_ge(pe, PAIR_DONE(PPB * k + PPB - 1))
                    vector.tensor_tensor(
                        out=e_sb[:, k].rearrange("p (r f) -> p r f", r=PPB),
                        in0=q_ps[k][:, 0:QW].rearrange("p (r f) -> p r f", r=PPB),
                        in1=cd2r,
                        op=mybir.AluOpType.mult,
                    ).then_inc(dve, 1)
                vector.wait_ge(dve, 3 * g + 2)  # own mults retired
                vector.tensor_reduce(
                    out=r_sb[:, 2 * g * PPB : (2 * g + 2) * PPB],
                    in_=e_sb[:, 2 * g : 2 * g + 2].rearrange(
                        "p b (r f) -> p (b r) f", r=PPB
                    ),
                    axis=mybir.AxisListType.X,
                    op=mybir.AluOpType.add,
                ).then_inc(dve, 1)


    return nc


def _np_mm_dtype():
    if USE_BF16:
        import ml_dtypes

        return ml_dtypes.bfloat16
    return np.float32


def make_consts(jastrow_coeff: np.ndarray):
    C = np.asarray(jastrow_coeff, dtype=np.float32).reshape(Nb, Nb)
    cp = np.zeros((128, CP_COLS), dtype=np.float32)
    bd2 = np.zeros((64, 64), dtype=np.float32)
    for i in range(2):
        bd2[32 * i : 32 * (i + 1), 32 * i : 32 * (i + 1)] = C / NPAIR
    for r in range(CP_COLS // 64):
        cp[0:64, 64 * r : 64 * (r + 1)] = bd2
        cp[64:128, 64 * r : 64 * (r + 1)] = bd2
    return cp


def shard_x(basis_single_body: np.ndarray):
    x = np.asarray(basis_single_body, dtype=np.float32)
    xt = np.ascontiguousarray(x.transpose(1, 0, 2))  # [N, B, Nb]
    dt = _np_mm_dtype()
    out = []
    for m in range(NCORES):
        sl = xt[:, m * BS : (m + 1) * BS, :]
        a = sl[:, 0:HB, :].reshape(N, FREE)
        b = sl[:, HB:BS, :].reshape(N, FREE)
        out.append(np.ascontiguousarray(np.concatenate([a, b], axis=0)).astype(dt))
    return out


def unpack_j(j: np.ndarray) -> np.ndarray:
    """j[i, p] -> per-core J[48]: col p is pair p; blocks 0,1 = half A batch
    2p+i, blocks 2,3 = half B batch 24+2p+(i-2)."""
    j = np.asarray(j, dtype=np.float32)
    ja = j[0:2, :].T.ravel()
    jb = j[2:4, :].T.ravel()
    return np.concatenate([ja, jb]).astype(np.float32)


_NC_CACHE: list = []


def kernel(basis_single_body: np.ndarray, jastrow_coeff: np.ndarray) -> np.ndarray:
    if not _NC_CACHE:
        _NC_CACHE.append(build_nc())
    nc = _NC_CACHE[0]

    cp = make_consts(jastrow_coeff)
    shards = shard_x(basis_single_body)
    in_maps = [{"x": s, "cp": cp} for s in shards]

    res = run_bass_kernel_spmd(nc, in_maps, core_ids=list(range(NCORES)))
    return np.concatenate([unpack_j(np.asarray(r["j"])) for r in res.results])


# revision 35
# speedup vs baseline: 1.0266x; 1.0266x over previous
"""Trainium2 Bass kernel for nn_BasisJastrow.

Math (per batch element b):
    J_b = (1/P) * sum_{i<j} chi_j^T C chi_i ,   P = N(N-1)/2, C = coeff.reshape(Nb, Nb)

Device decomposition (per core; data-parallel over the batch axis).
The 48 per-core batches are split into two halves on disjoint SBUF partition
ranges (A = batches 0..23 on partitions 0:64, B = batches 24..47 on 64:128).
All matmul tensors are bf16 (PSUM accumulation stays f32); tolerance is 2e-2.

  layout  Xl[n + 64*half, (b,u)]   n=64 on partitions, 24 batches * 32 per half
  phase 1 S  = Lt.T @ Xl           exclusive prefix sums over particles (PE,
                                   concurrent 64x64 quadrants per half,
                                   4 x 192-column sub-chunks)
  phase 2 Q_p = Xp.T @ Sp          2-batch cross-Gram [64,64] per half; diag
                                   32x32 blocks are G_b, off-diag is garbage
  phase 3 r_p[q] = sum_f Q_p[q,f] * CD2[q,f]  DVE multiplies each 3-pair
                                   bank by the mask (PSUM -> SBUF bf16) and
                                   reduces bank pairs with tensor_reduce
  phase 4 J = id4.T @ R            partition-block reduction of r (PE)

Engine roles: Sync = ring A DMA (half A + const pack) + output DMA.
Scalar = ring B DMA + the four S casts (a dummy activation preloads the
1.3us ACT_TABLE during the DMA window).  GpSimd = Lt/id4 generation.
DVE = mask mults + reductions + J copy.  PE = warmup + phases 1/2/4.

DMA cost is per dst-partition line (~10ns/line), so each ring loads its
64-partition half in ONE transfer; column chunking would double the time.

Raw Bass (explicit engine blocks + semaphores): the walrus build in this
container rejects any instruction carrying more than one sync wait, which
rules out Tile's generated sem placement; raw Bass emits waits standalone.
"""

import sys

for _p in ("/opt/trn_rl_repo",):
    if _p not in sys.path:
        sys.path.insert(0, _p)

import numpy as np

import concourse.bass as bass
from concourse import mybir
from concourse.bass_utils import run_bass_kernel_spmd

B, N, Nb = 384, 64, 32
NCORES = 8
BS = B // NCORES            # 48 batches per core
HB = BS // 2                # 24 batches per half
NP = HB // 2                # 12 concurrent gram pairs
NPAIR = N * (N - 1) // 2    # 2016
F32 = mybir.dt.float32

MM_DTYPE = "bf16"  # "f32" | "bf16"
USE_BF16 = MM_DTYPE == "bf16"
MM_DT = mybir.dt.bfloat16 if USE_BF16 else mybir.dt.float32

FREE = HB * Nb              # 768 free columns per half
NSUB = 4                    # phase-1 / cast sub-chunks (192 cols each)
SUB = FREE // NSUB
NBANK = 4                   # gram psum banks
PPB = 3                     # pairs per bank
N_WARM = 6                  # gapless PE warmup matmuls during the DMA window
WARM_COLS = 192

CP_COLS = 192               # const pack = CD2 tiled 3x (one bank's pairs)


def build_nc() -> bass.Bass:
    nc = bass.Bass()

    x_d = nc.dram_tensor("x", [128, FREE], MM_DT, kind="ExternalInput")
    cp_d = nc.dram_tensor("cp", [128, CP_COLS], F32, kind="ExternalInput")
    j_d = nc.dram_tensor("j", [4, NP], F32, kind="ExternalOutput")

    from contextlib import ExitStack

    with ExitStack() as ctx:
        ctx.enter_context(
            nc.allow_low_precision("bf16 pair sums stay within the 2e-2 gate")
        )
        x_sb = ctx.enter_context(nc.sbuf_tensor("x_sb", [128, FREE], MM_DT))
        s_sb = ctx.enter_context(nc.sbuf_tensor("s_sb", [128, FREE], MM_DT))
        cp_sb = ctx.enter_context(nc.sbuf_tensor("cp_sb", [128, CP_COLS], F32))
        w_sb = ctx.enter_context(
            nc.sbuf_tensor("w_sb", [128, 4 + WARM_COLS], MM_DT)
        )
        lt_sb = ctx.enter_context(nc.sbuf_tensor("lt_sb", [128, N], MM_DT))
        id4_sb = ctx.enter_context(nc.sbuf_tensor("id4_sb", [128, 4], MM_DT))
        dum_sb = ctx.enter_context(nc.sbuf_tensor("dum_sb", [128, 1], MM_DT))
        e_sb = ctx.enter_context(
            nc.sbuf_tensor("e_sb", [128, NBANK, PPB * 64], MM_DT)
        )
        r_sb = ctx.enter_context(nc.sbuf_tensor("r_sb", [128, NP], MM_DT))
        j_sb = ctx.enter_context(nc.sbuf_tensor("j_sb", [4, NP], F32))
        s_ps = [
            ctx.enter_context(
                nc.psum_tensor(f"s_ps{h}", [128, FREE // 2], F32)
            )
            for h in range(2)
        ]
        QW = PPB * 64
        q_ps = [
            ctx.enter_context(
                nc.psum_tensor(
                    f"q_ps{k}", [128, QW + (16 if k == 0 else 0)], F32
                )
            )
            for k in range(NBANK)
        ]

        def sps(cs):
            h, off = cs.start // (FREE // 2), cs.start % (FREE // 2)
            return s_ps[h][:, off : off + (cs.stop - cs.start)]
        jw = None  # set below once q_ps exists
        dma_a = ctx.enter_context(nc.semaphore("dma_a"))
        dma_b = ctx.enter_context(nc.semaphore("dma_b"))
        dma_c = ctx.enter_context(nc.semaphore("dma_c"))
        dma_o = ctx.enter_context(nc.semaphore("dma_o"))
        pe = ctx.enter_context(nc.semaphore("pe"))
        pw = ctx.enter_context(nc.semaphore("pw"))
        dve = ctx.enter_context(nc.semaphore("dve"))
        sc = ctx.enter_context(nc.semaphore("sc"))
        gp_w = ctx.enter_context(nc.semaphore("gp_w"))
        block = ctx.enter_context(nc.Block())
        jw = q_ps[0][0:4, QW : QW + NP]

        lt = lt_sb[:]
        cd2r = cp_sb[:, 0:CP_COLS].rearrange("p (r f) -> p r f", r=PPB)

        # pe ledger: phase1 sub-chunks -> 1..NSUB; pair p done -> NSUB+1+p;
        # phase4 halves -> NSUB+NP+1, +2
        PAIR_DONE = lambda p: NSUB + 1 + p
        PH4_DONE = NSUB + NP + 1
        # dve ledger: m0,m1 -> 1,2; r01 -> 3; m2,m3 -> 4,5; r23 -> 6;
        # j halves -> 7,8
        JCOPY_DONE = 7
        # sc ledger: 1..NSUB = S casts

        @block.sync
        def _(sync):
            # ring A: half A (partitions 0:64) in one transfer, then consts
            sync.dma_start(out=x_sb[0:64, :], in_=x_d[0:64, :]).then_inc(
                dma_a, 16
            )
            sync.dma_start(out=cp_sb[:], in_=cp_d[:]).then_inc(dma_c, 16)
            # J leaves in two halves; the first DMA trigger (~0.9us) runs
            # while the DVE still works on banks 2/3
            for g in range(2):
                sync.wait_ge(sc, NSUB + 1 + g)
                cols = slice(2 * PPB * g, 2 * PPB * (g + 1))
                sync.dma_start(out=j_d[:, cols], in_=j_sb[:, cols]).then_inc(
                    dma_o, 16
                )

        @block.scalar
        def _(scalar):
            # ring B: half B (partitions 64:128) in one transfer
            scalar.dma_start(out=x_sb[64:128, :], in_=x_d[64:128, :]).then_inc(
                dma_b, 16
            )
            # dummy activation: hides the one-time ACT_TABLE_LOAD (~1.5us)
            # inside the DMA window so the S casts below start immediately
            scalar.wait_ge(gp_w, 1)
            scalar.copy(dum_sb[:], w_sb[:, 0:1])
            # S casts (PSUM f32 -> SBUF bf16), one per phase-1 sub-chunk
            for c in range(NSUB):
                cs = slice(c * SUB, (c + 1) * SUB)
                scalar.wait_ge(pe, c + 1)
                scalar.copy(s_sb[:, cs], sps(cs)).then_inc(sc, 1)
            # J copies (PSUM -> SBUF) per half, handed to Sync's out DMA
            for g in range(2):
                cols = slice(2 * PPB * g, 2 * PPB * (g + 1))
                scalar.wait_ge(pe, PH4_DONE + g)
                scalar.copy(j_sb[:, cols], jw[:, cols]).then_inc(sc, 1)

        @block.gpsimd
        def _(gpsimd):
            gpsimd.memset(w_sb[:], 1.0).then_inc(gp_w, 1)
            gpsimd.wait_ge(gp_w, 1)
            for h in range(2):
                hs = slice(h * 64, (h + 1) * 64)
                gpsimd.affine_select(
                    out=lt_sb[hs, :],
                    in_=w_sb[hs, 0:N],
                    pattern=[[1, N]],
                    compare_op=mybir.AluOpType.is_gt,
                    fill=0.0,
                    base=0,
                    channel_multiplier=-1,
                ).then_inc(gp_w, 1)
            # id4 for phase 4: col k = ones on partitions 32k:32k+32.
            # Disjoint 32-partition memsets (race checker + GpSimd window)
            for k in range(4):
                for blk in range(4):
                    inst = gpsimd.memset(
                        id4_sb[32 * blk : 32 * (blk + 1), k : k + 1],
                        1.0 if blk == k else 0.0,
                    )
            inst.then_inc(gp_w, 1)

        @block.tensor
        def _(tensor):
            # gapless PE warmups (disjoint s_ps scratch regions, so no
            # chain sems) keep the PE p-state high through the DMA window
            tensor.wait_ge(gp_w, 1)
            for w in range(N_WARM):
                if w < 4:
                    tgt = s_ps[w // 2][
                        0:4, (w % 2) * WARM_COLS : (w % 2 + 1) * WARM_COLS
                    ]
                else:
                    tgt = q_ps[w - 4][0:4, 0:WARM_COLS]
                inst = tensor.matmul(
                    tgt,
                    w_sb[0:64, 0:4],
                    w_sb[0:64, 4 : 4 + WARM_COLS],
                    start=True,
                    stop=True,
                )
            inst.then_inc(pw, 1)
            tensor.wait_ge(gp_w, 3)
            # order phase 1's s_ps writes after the warm scratch writes
            tensor.wait_ge(pw, 1)
            # phase 1: exclusive prefix sums, concurrent halves
            for c in range(NSUB):
                cs = slice(c * SUB, (c + 1) * SUB)
                if c == 0:
                    tensor.wait_ge(dma_a, 16)
                sp = sps(cs)
                tensor.matmul(
                    sp[0:64, :],
                    lt[0:64, :],
                    x_sb[0:64, cs],
                    start=True,
                    stop=True,
                    tile_position=(0, 0),
                )
                if c == 0:
                    tensor.wait_ge(dma_b, 16)
                tensor.matmul(
                    sp[64:128, :],
                    lt[64:128, :],
                    x_sb[64:128, cs],
                    start=True,
                    stop=True,
                    tile_position=(64, 64),
                ).then_inc(pe, 1)
            # phase 2: 2-batch cross-Grams, concurrent halves
            for p in range(NP):
                ps_ = slice(p * 64, (p + 1) * 64)
                if p % PPB == 0:
                    tensor.wait_ge(sc, (p * 64) // SUB + 1)
                q = q_ps[p // PPB][:, (p % PPB) * 64 : (p % PPB + 1) * 64]
                tensor.matmul(
                    q[0:64, :],
                    x_sb[0:64, ps_],
                    s_sb[0:64, ps_],
                    start=True,
                    stop=True,
                    tile_position=(0, 0),
                )
                tensor.matmul(
                    q[64:128, :],
                    x_sb[64:128, ps_],
                    s_sb[64:128, ps_],
                    start=True,
                    stop=True,
                    tile_position=(64, 64),
                ).then_inc(pe, 1)
            # phase 4: partition-block reduction of r columns, split in
            # two so the first half overlaps the rest of the DVE chain
            tensor.wait_ge(gp_w, 4)
            for g in range(2):
                cols = slice(2 * PPB * g, 2 * PPB * (g + 1))
                tensor.wait_ge(dve, 3 * (g + 1))
                tensor.matmul(
                    jw[:, cols],
                    id4_sb[:],
                    r_sb[:, cols],
                    start=True,
                    stop=True,
                ).then_inc(pe, 1)

        @block.vector
        def _(vector):
            # phase 3 mask-mults: e = Q * CD2 per 3-pair bank (PSUM->SBUF),
            # with a fused 2-bank reduce after each pair of mults
            vector.wait_ge(dma_c, 16)
            for g in range(2):
                for k in (2 * g, 2 * g + 1):
                    vector.wait_ge(pe, PAIR_DONE(PPB * k + PPB - 1))
                    vector.tensor_tensor(
                        out=e_sb[:, k].rearrange("p (r f) -> p r f", r=PPB),
                        in0=q_ps[k][:, 0:QW].rearrange("p (r f) -> p r f", r=PPB),
                        in1=cd2r,
                        op=mybir.AluOpType.mult,
                    ).then_inc(dve, 1)
                vector.wait_ge(dve, 3 * g + 2)  # own mults retired
                vector.tensor_reduce(
                    out=r_sb[:, 2 * g * PPB : (2 * g + 2) * PPB],
                    in_=e_sb[:, 2 * g : 2 * g + 2].rearrange(
                        "p b (r f) -> p (b r) f", r=PPB
                    ),
                    axis=mybir.AxisListType.X,
                    op=mybir.AluOpType.add,
                ).then_inc(dve, 1)


    return nc


def _np_mm_dtype():
    if USE_BF16:
        import ml_dtypes

        return ml_dtypes.bfloat16
    return np.float32


def make_consts(jastrow_coeff: np.ndarray):
    C = np.asarray(jastrow_coeff, dtype=np.float32).reshape(Nb, Nb)
    cp = np.zeros((128, CP_COLS), dtype=np.float32)
    bd2 = np.zeros((64, 64), dtype=np.float32)
    for i in range(2):
        bd2[32 * i : 32 * (i + 1), 32 * i : 32 * (i + 1)] = C / NPAIR
    for r in range(CP_COLS // 64):
        cp[0:64, 64 * r : 64 * (r + 1)] = bd2
        cp[64:128, 64 * r : 64 * (r + 1)] = bd2
    return cp


def shard_x(basis_single_body: np.ndarray):
    x = np.asarray(basis_single_body, dtype=np.float32)
    xt = np.ascontiguousarray(x.transpose(1, 0, 2))  # [N, B, Nb]
    dt = _np_mm_dtype()
    out = []
    for m in range(NCORES):
        sl = xt[:, m * BS : (m + 1) * BS, :]
        a = sl[:, 0:HB, :].reshape(N, FREE)
        b = sl[:, HB:BS, :].reshape(N, FREE)
        out.append(np.ascontiguousarray(np.concatenate([a, b], axis=0)).astype(dt))
    return out


def unpack_j(j: np.ndarray) -> np.ndarray:
    """j[i, p] -> per-core J[48]: col p is pair p; blocks 0,1 = half A batch
    2p+i, blocks 2,3 = half B batch 24+2p+(i-2)."""
    j = np.asarray(j, dtype=np.float32)
    ja = j[0:2, :].T.ravel()
    jb = j[2:4, :].T.ravel()
    return np.concatenate([ja, jb]).astype(np.float32)


_NC_CACHE: list = []


def kernel(basis_single_body: np.ndarray, jastrow_coeff: np.ndarray) -> np.ndarray:
    if not _NC_CACHE:
        _NC_CACHE.append(build_nc())
    nc = _NC_CACHE[0]

    cp = make_consts(jastrow_coeff)
    shards = shard_x(basis_single_body)
    in_maps = [{"x": s, "cp": cp} for s in shards]

    res = run_bass_kernel_spmd(nc, in_maps, core_ids=list(range(NCORES)))
    return np.concatenate([unpack_j(np.asarray(r["j"])) for r in res.results])
